# revision 24
# baseline (speedup 1.0000x reference)
"""Tensor-parallel GQA multi-head attention (RoPE + causal softmax) for 8 trn2 cores.

Sharding: every core handles BOTH batches with 4 q-heads / 1 kv-head:
core c owns q-heads {4c..4c+3} (kv-head c) of batches 0 and 1. Attention
runs in transposed (feature-major) layout with flash-style causal tiling.
Per 512-token slab, the 8 cores exchange their normalized attention outputs
with AllToAlls (bf16) so that core c ends up with ALL 2048 attention
features for its 128-position output stripe (batch c//4, stripe c%4); it
then applies the full wo to produce disjoint output rows. No reduction
collective is needed.

v4 notes (vs the 502us baseline):
- All bulk inputs are host-prearranged so every DMA reads contiguous
  per-partition rows (big descriptors); DMAs are batched via 2D/3D access
  patterns. Each dma_start costs ~600ns of issuing-engine time, and 256B
  descriptors cap SDMA throughput at ~60GB/s, so both count and layout
  matter.
- Causal mask via one DVE bf16 multiply per diagonal tile (both heads in a
  [128,2,128] AP) instead of gpsimd affine_selects.
- RoPE half-swap via DVE stream_shuffle: head-dims are host-permuted (P64)
  so the rotate_half partner sits 16 lanes away within each 32-lane block
  (scores are invariant to a consistent q/k feature permutation).
- Attention output per (b,t) is kept as [64, 1024] (two heads side by side
  in the free dim) so psum evacuation is partition-aligned DVE work only.
- The per-slab AllToAll is split by t: the t=0 half kicks in the middle of
  the slab (fully hidden); only the t=1 half (256KB) lands at the slab
  boundary, and the last slab's wo t0-half runs while it is in flight,
  which also keeps the PE HAM-warm through the tail.
"""

import sys

sys.path.insert(0, "/opt/trn_rl_repo")

import numpy as np

import concourse.bass as bass
import concourse.bacc as bacc
import concourse.mybir as mybir
from concourse import tile
from concourse.bass_utils import run_bass_kernel_spmd

B, S, D = 2, 2048, 2048
N_HEADS, N_KV, HD = 32, 8, 64
NCORES = 8
QH = 4    # q-heads per core
FQ = QH * HD       # 256 q-feature cols per core
FKV = 2 * HD       # 128 (K then V) per core
SCALE = 1.0 / 8.0  # 1/sqrt(HD)

QTILE = 512
KTILE = 128
NSLAB = S // QTILE  # 4
ND = D // 128       # 16 contraction chunks

F32 = mybir.dt.float32
EXP = mybir.ActivationFunctionType.Exp
BF16 = mybir.dt.bfloat16
MMD = BF16

# rope partner sits 16 lanes away inside each 32-lane block (see P64 in
# _host_inputs)
SHUF_MASK = [(i + 16) % 32 for i in range(32)]


def _build_kernel(tc, io):
    nc = tc.nc
    xP, wq, wkv, wo = io["xP"], io["wq"], io["wkv"], io["wo"]
    cos2, sin2s, selc, tri = io["cos2"], io["sin2s"], io["selc"], io["tri"]
    out_full = io["out"]
    single = bool(io.get("single"))

    # ---------------- pools ----------------
    const = tc.alloc_tile_pool(name="const", bufs=1)
    wpool = tc.alloc_tile_pool(name="wpool", bufs=1, side="right")
    kvp = tc.alloc_tile_pool(name="kvp", bufs=1)
    xpool = tc.alloc_tile_pool(name="xpool", bufs=2)
    qpool = tc.alloc_tile_pool(name="qpool", bufs=2)
    aop = tc.alloc_tile_pool(name="aop", bufs=2, side="right")
    dsp = tc.alloc_tile_pool(name="dsp", bufs=1)
    rp = tc.alloc_tile_pool(name="rp", bufs=2)
    pexp = tc.alloc_tile_pool(name="pexp", bufs=5)
    evac = tc.alloc_tile_pool(name="evac", bufs=2)
    aogp = tc.alloc_tile_pool(name="aogp", bufs=1, side="right")
    dram = tc.alloc_tile_pool(name="dram", bufs=1, space="DRAM")

    psM = tc.alloc_tile_pool(name="psM", bufs=2, space="PSUM")
    psS = tc.alloc_tile_pool(name="psS", bufs=2, space="PSUM")
    psO = tc.alloc_tile_pool(name="psO", bufs=1, space="PSUM")

    # ------- constants + weights; all contiguous per-partition rows -------
    def load_x(b, j):
        xts = []
        for h in range(2):
            xt = xpool.tile([128, 8 * QTILE], MMD, name="xt", tag=f"xt{h}")
            c0 = ((b * NSLAB + j) * ND + 8 * h) * QTILE
            nc.sync.dma_start(xt[:], xP[:, c0:c0 + 8 * QTILE])
            xts.append(xt)
        return xts

    xts00 = load_x(0, 0)

    WQ = []
    for f in range(2):
        w = wpool.tile([128, ND * 128], MMD, name=f"wqt{f}")
        nc.scalar.dma_start(w[:], wq[:, f * 2048:(f + 1) * 2048])
        WQ.append(w)
    WKV = wpool.tile([128, ND * 128], MMD, name="wkvt")
    nc.scalar.dma_start(WKV[:], wkv[:, :])

    cos2_t = const.tile([128, S], MMD)
    nc.sync.dma_start(cos2_t[:], cos2[:])
    sin2s_t = const.tile([128, S], MMD)
    nc.sync.dma_start(sin2s_t[:], sin2s[:])
    tri_t = const.tile([128, 2 * KTILE], MMD)
    nc.sync.dma_start(tri_t[:], tri[:])
    selc_t = const.tile([QH, 4 * 64], MMD)
    nc.sync.dma_start(selc_t[:], selc[:])

    ident = const.tile([128, 64], F32)
    nc.gpsimd.memset(ident[:], 0.0)
    for p in (0, 64):
        nc.gpsimd.affine_select(
            out=ident[p:p + 64, :], in_=ident[p:p + 64, :],
            compare_op=mybir.AluOpType.not_equal,
            fill=1.0, base=0, pattern=[[-1, 64]], channel_multiplier=1,
        )

    # full wo in one tile (64KB per partition, contiguous rows), loaded in
    # 4 staggered chunks so the 8MB does not crowd out the startup x loads
    WO = wpool.tile([128, ND * D], MMD, name="wot")

    def load_wo_chunk(i):
        nc.scalar.dma_start(WO[:, i * 4 * D:(i + 1) * 4 * D],
                            wo[:, i * 4 * D:(i + 1) * 4 * D])
    load_wo_chunk(0)

    # persistent K/V cache tiles
    KK = [kvp.tile([128, S], MMD, name=f"kk{b}") for b in range(B)]
    VA = {}
    for b in range(B):
        for i in range(S // KTILE):
            VA[b, i] = kvp.tile([128, HD + 1], MMD, name=f"va{b}_{i}")

    # A2A dram tiles: one pair per (slab, t); chunk for dest core d=4b+g is
    # rows [128d:128d+128] = (h, p) with the wo feature order fc = 2*cc + t
    a2a_in = [[dram.tile([128 * NCORES, KTILE], MMD, name=f"ain{j}_{t}")
               for t in range(2)] for j in range(NSLAB)]
    a2a_out = [[dram.tile([128 * NCORES, KTILE], MMD, name=f"aout{j}_{t}")
                for t in range(2)] for j in range(NSLAB)]

    AO = {}   # per (b, t): [64, 1024] attention out, heads side by side
    QT = {}

    def rope(dst, rows, qs, tab_qs, dup_hi=False):
        # dst[rows, qs] = dst*cos + shuffle16(dst)*sin  (feature-major RoPE;
        # the host-side P64 head-dim permutation makes the rotate_half
        # partner a +16 lane rotation within each 32-lane block). With
        # dup_hi, the roped result is also written to partitions 64:128
        # (64-partition DVE ops may write either half).
        n = rows[1] - rows[0]
        sl = (slice(rows[0], rows[1]), qs)
        qsw = rp.tile([128, QTILE], MMD, name="qsw", tag="qsw")
        nc.vector.stream_shuffle(qsw[:n], dst[sl], mask=SHUF_MASK)
        t1 = rp.tile([128, QTILE], MMD, name="t1", tag="t1")
        nc.vector.tensor_mul(t1[:n], dst[sl], cos2_t[rows[0]:rows[1], tab_qs])
        t2 = rp.tile([128, QTILE], MMD, name="t2", tag="t2")
        nc.vector.tensor_mul(t2[:n], qsw[:n], sin2s_t[rows[0]:rows[1], tab_qs])
        nc.vector.tensor_add(dst[sl], t1[:n], t2[:n])
        if dup_hi:
            nc.vector.tensor_add(dst[64:64 + n, qs], t1[:n], t2[:n])

    def make_proj_fillers(b, j, xts):
        # projection for (b, j), split into small PE chunks so it can be
        # woven into the preceding attention's exp-bound inner loop
        qs = slice(j * QTILE, (j + 1) * QTILE)
        ctx = {}
        fillers = []
        if xts is not None:
            ctx["x"] = xts
        else:
            def loadx():
                ctx["x"] = load_x(b, j)
            fillers.append(loadx)
        for f in (0, 2, 1):
            for sub in range(8):
                def mmchunk(f=f, sub=sub):
                    if sub == 0:
                        ctx[f] = psM.tile([128, QTILE], F32, name="psq",
                                          tag="mm")
                    ps = ctx[f]
                    for k in range(2 * sub, 2 * sub + 2):
                        w = WQ[f] if f < 2 else WKV
                        xt = ctx["x"][k // 8]
                        nc.tensor.matmul(
                            ps[:], w[:, k * 128:(k + 1) * 128],
                            xt[:, (k % 8) * QTILE:(k % 8 + 1) * QTILE],
                            start=(k == 0), stop=(k == ND - 1))
                fillers.append(mmchunk)

            def evacf(f=f):
                ps = ctx[f]
                if f < 2:
                    qt = qpool.tile([128, QTILE], MMD, name="qt",
                                    tag=f"qt{b}_{f}")
                    QT[b, f] = qt
                    nc.vector.tensor_copy(qt[:], ps[:])
                    rope(qt, (0, 128), slice(0, QTILE), qs)
                else:
                    nc.vector.tensor_copy(KK[b][0:64, qs], ps[0:64, :])
                    rope(KK[b], (0, 64), qs, qs, dup_hi=True)
                    vv = rp.tile([128, QTILE], F32, name="vv", tag="vv")
                    nc.vector.tensor_copy(vv[64:128, :], ps[64:128, :])
                    ctx["vv"] = vv
            fillers.append(evacf)
        for c in range(4):
            def vtrans(c=c):
                i = 4 * j + c
                tp = psM.tile([128, QTILE], F32, name="tp", tag="mm")
                vv = ctx["vv"]
                nc.tensor.matmul(tp[:, 0:HD],
                                 vv[64:128, c * 128:(c + 1) * 128],
                                 ident[64:128, :], is_transpose=True,
                                 start=True, stop=True)
                va = VA[b, i]
                nc.vector.tensor_copy(va[:, 0:HD], tp[:, 0:HD])
                nc.vector.memset(va[:, HD:HD + 1], 1.0)
            fillers.append(vtrans)
        return fillers

    def proj(b, j, xts=None):
        for f in make_proj_fillers(b, j, xts):
            f()

    def attn(b, j, t, fillers=None):
        # one (batch, head-pair) attention pass over slab j. fillers:
        # closures emitting small independent PE chunks (wo/proj work);
        # paced evenly and placed before each attnV pair so the tensor
        # engine has work while it would otherwise stall on the exp
        fillers = list(fillers or [])
        nkt = 4 * j + 4
        rate = len(fillers) / max(1.0, 0.75 * nkt)
        acc = [0.0]

        def pop_fillers():
            acc[0] += rate
            while fillers and acc[0] >= 1.0:
                fillers.pop(0)()
                acc[0] -= 1.0
        ds = AO["ds"]
        oA = psO.tile([HD + 1, QTILE], F32, name="oA", tag="oA")
        oB = psO.tile([HD + 1, QTILE], F32, name="oB", tag="oB")
        sabs = {}

        def scores(i):
            r = i - 4 * j
            off = max(r, 0) * KTILE
            ks = slice(i * KTILE, (i + 1) * KTILE)
            sAB = psS.tile([128, 2 * QTILE], F32, name="sAB", tag="sAB")
            nc.tensor.matmul(sAB[:, off:QTILE], KK[b][0:64, ks],
                             QT[b, t][0:64, off:], start=True, stop=True,
                             tile_position=(0, 0))
            nc.tensor.matmul(sAB[:, QTILE + off:], KK[b][64:128, ks],
                             QT[b, t][64:128, off:], start=True, stop=True,
                             tile_position=(64, 0))
            sabs[i] = sAB

        scores(0)
        for i in range(nkt):
            r = i - 4 * j
            off = max(r, 0) * KTILE
            if i + 1 < nkt:
                scores(i + 1)
            sAB = sabs.pop(i)
            pAB = pexp.tile([128, 2 * QTILE], MMD, name="pAB", tag="pAB")
            nc.scalar.activation(pAB[:, off:], sAB[:, off:], EXP,
                                 scale=SCALE)
            if r >= 0:
                # zero the strictly-upper triangle (causal mask) of the
                # diagonal block for both heads in one DVE multiply
                pv = pAB[:].rearrange("p (h q) -> p h q", h=2)
                pv = pv[:, :, off:off + KTILE]
                tv = tri_t[:].rearrange("p (h q) -> p h q", h=2)
                nc.vector.tensor_mul(pv, pv, tv)
            pop_fillers()
            nc.tensor.matmul(oA[:, off:], VA[b, i][:], pAB[:, off:QTILE],
                             start=(i == 0), stop=(i == nkt - 1))
            nc.tensor.matmul(oB[:, off:], VA[b, i][:], pAB[:, QTILE + off:],
                             start=(i == 0), stop=(i == nkt - 1))
        # evacuate into the AO layout: heads side by side in the free dim
        # (all partition-aligned); denominator rows (partition 64) via
        # aligned ScalarE copies into the ds staging row
        ao = AO[b, t]
        nc.vector.tensor_copy(ao[:, 0:QTILE], oA[0:64, :])
        nc.vector.tensor_copy(ao[:, QTILE:2 * QTILE], oB[0:64, :])
        for h, o in ((0, oA), (1, oB)):
            c0 = t * 4 * QTILE + (2 * b + h) * QTILE
            nc.scalar.copy(ds[64:65, c0:c0 + QTILE], o[64:65, :])
        for f in fillers:
            f()

    def finish(j, t, AOj):
        # normalize the two (b, *) pairs of this t and kick their A2A half
        ds = AOj["ds"]
        dn = evac.tile([QH, QTILE], MMD, name="dn", tag="dn")
        nc.sync.dma_start(
            dn[:, :], ds[64:65, t * 4 * QTILE:(t + 1) * 4 * QTILE])
        dnF = evac.tile([QH, QTILE], F32, name="dnF", tag="dnF", bufs=1)
        nc.vector.tensor_copy(dnF[:], dn[:])
        dnR = evac.tile([QH, QTILE], F32, name="dnR", tag="dnR", bufs=1)
        nc.vector.reciprocal_approx_fast(out=dnR[:], in_=dnF[:])
        dnRb = evac.tile([QH, QTILE], MMD, name="dnRb", tag="dnRb")
        nc.vector.tensor_copy(dnRb[:], dnR[:])
        # a2a row for (b, h, g): 128*(4b+g) + 64h + p
        dst_all = a2a_in[j][t][:, :].rearrange(
            "(bb g h r) q -> bb h r g q", bb=2, g=4, h=2)
        for b in range(B):
            ao = AOj[b, t]
            for h in range(2):
                u = 2 * b + h
                bc = psM.tile([128, QTILE], F32, name="bc", tag="mm")
                nc.tensor.matmul(
                    bc[0:64, :], selc_t[:, u * 64:(u + 1) * 64],
                    dnRb[:], start=True, stop=True)
                nc.vector.tensor_mul(ao[:, h * QTILE:(h + 1) * QTILE],
                                     ao[:, h * QTILE:(h + 1) * QTILE],
                                     bc[0:64, :])
            for h in range(2):
                nc.sync.dma_start(
                    dst_all[b:b + 1, h:h + 1],
                    ao[:, h * QTILE:(h + 1) * QTILE].rearrange(
                        "p (g q) -> p g q", g=4))
        if single:
            nc.sync.dma_start(a2a_out[j][t][:], a2a_in[j][t][:])
        else:
            nc.gpsimd.collective_compute(
                "AllToAll", mybir.AluOpType.bypass,
                replica_groups=[list(range(NCORES))],
                ins=[a2a_in[j][t][:]], outs=[a2a_out[j][t][:]],
            )

    def make_wo_fillers(j, tail=False):
        # wo for slab j. Feature chunk fc = 2*cc + t lives in a2a_out[j][t]
        # rows [128cc:128cc+128]. In tail mode the gathers go on the gpsimd
        # queue (so they cannot head-of-line-block the sync queue) and the
        # t=0 half of the accumulation runs while the t=1 A2A is in flight.
        ctx = {}

        def gather(t, q=None):
            aog = aogp.tile([128, NCORES * KTILE], MMD, name="aog",
                            tag=f"aog{t}")
            (q or nc.sync).dma_start(aog[:], a2a_out[j][t][:, :].rearrange(
                "(k p) q -> p k q", p=128))
            ctx["aog", t] = aog

        def chunk(dn_, pos, cc, t, npos):
            if pos == 0:
                ctx[dn_] = psM.tile([128, QTILE], F32, name="psW", tag="mm")
            ps = ctx[dn_]
            aog = ctx["aog", t]
            fc = 2 * cc + t
            nc.tensor.matmul(
                ps[:], aog[:, cc * KTILE:(cc + 1) * KTILE],
                WO[:, fc * D + dn_ * QTILE:fc * D + (dn_ + 1) * QTILE],
                start=(pos == 0), stop=(pos == npos - 1))

        def evacf(dn_):
            og = evac.tile([128, QTILE], F32, name="og", tag="og", bufs=3)
            nc.vector.tensor_copy(og[:], ctx[dn_][:])
            nc.gpsimd.dma_start(
                out_full[j * 128:(j + 1) * 128,
                         dn_ * QTILE:(dn_ + 1) * QTILE], og[:])

        fillers = []
        if not tail:
            fillers.append(lambda: (gather(0), gather(1)))
            for dn_ in range(4):
                for sub in range(8):
                    def w(dn_=dn_, sub=sub):
                        for k in range(2):
                            pos = 2 * sub + k
                            cc, t = pos % 8, pos // 8
                            chunk(dn_, pos, cc, t, 16)
                        if sub == 7:
                            evacf(dn_)
                    fillers.append(w)
        else:
            # two-round accumulation: the t0 half completes into a bf16
            # partial (covering the in-flight t=1 A2A with dense PE work);
            # the t1 half accumulates fresh and a DVE add merges them.
            # fillers[0] (the t0 gather) is pre-issued right after the t0
            # collective is triggered.
            fillers.append(lambda: gather(0, nc.gpsimd))
            fillers.append(lambda: gather(1, nc.gpsimd))

            def t0round(dn_):
                chunk(dn_, 0, 0, 0, 8)
                for cc in range(1, 8):
                    chunk(dn_, cc, cc, 0, 8)
                og2 = ctx.setdefault(
                    "og2", evac.tile([128, D], MMD, name="og2", tag="og2",
                                     bufs=1))
                nc.vector.tensor_copy(
                    og2[:, dn_ * QTILE:(dn_ + 1) * QTILE], ctx[dn_][:])
                ctx.pop(dn_)

            def t1round(dn_):
                for cc in range(8):
                    chunk(dn_, cc, cc, 1, 8)
                og2 = ctx["og2"]
                og = evac.tile([128, QTILE], F32, name="og", tag="og",
                               bufs=3)
                nc.vector.tensor_add(
                    og[:], ctx[dn_][:], og2[:, dn_ * QTILE:(dn_ + 1) * QTILE])
                nc.gpsimd.dma_start(
                    out_full[j * 128:(j + 1) * 128,
                             dn_ * QTILE:(dn_ + 1) * QTILE], og[:])

            for dn_ in range(4):
                fillers.append(lambda dn_=dn_: t0round(dn_))
            for dn_ in range(4):
                fillers.append(lambda dn_=dn_: t1round(dn_))
        return fillers

    def wo_slab(j, tail=False):
        for f in make_wo_fillers(j, tail=tail):
            f()

    nxt1 = load_x(1, 0)
    proj(0, 0, xts00)
    for j in range(NSLAB - 1):
        AO.clear()
        AO["ds"] = dsp.tile([65, 8 * QTILE], MMD, name="ds", tag="ds")
        for b in range(B):
            for t in range(2):
                AO[b, t] = aop.tile([64, 2 * QTILE], MMD, name=f"ao{b}{t}",
                                    tag=f"ao{b}{t}")
        # weave proj(1, j) into attn(0, j)'s exp-bound loops; at j=0 run the
        # first matmul chunks immediately so the PE has work while the
        # slab-0 ropes drain through the DVE
        p1 = make_proj_fillers(1, j, nxt1)
        if j == 0:
            for f in p1[:9]:
                f()
            p1 = p1[9:]
        h = (len(p1) + 1) // 2
        attn(0, j, 0, fillers=p1[:h])
        if j == 0:
            load_wo_chunk(0)
        attn(0, j, 1, fillers=p1[h:])
        if j == 0:
            load_wo_chunk(1)
        # weave the next slab's proj(0) into attn(1, j, 0) and the previous
        # slab's wo into attn(1, j, 1)
        if j + 1 < NSLAB:
            nxt0 = load_x(0, j + 1)
            f_pj = make_proj_fillers(0, j + 1, nxt0)
        else:
            f_pj = []
        attn(1, j, 0, fillers=f_pj)
        if j == 0:
            load_wo_chunk(2)
        # pull the wo gather + first chunks ahead of finish(j,0) so the PE
        # has work while the broadcast matmuls wait on the reciprocal chain
        f_wo = make_wo_fillers(j - 1) if j > 0 else []
        for f in f_wo[:4]:
            f()
        finish(j, 0, AO)
        attn(1, j, 1, fillers=f_wo[4:])
        nxt1 = load_x(1, j + 1) if j + 1 < NSLAB else None
        if j == 0:
            load_wo_chunk(3)
        finish(j, 1, AO)

    # ---- last slab, t-major: both t=0 passes first so the t0 exchange is
    # in flight while the t=1 attention still runs ----
    j = NSLAB - 1
    AO.clear()
    AO["ds"] = dsp.tile([65, 8 * QTILE], MMD, name="ds", tag="ds")
    for b in range(B):
        for t in range(2):
            AO[b, t] = aop.tile([64, 2 * QTILE], MMD, name=f"ao{b}{t}",
                                tag=f"ao{b}{t}")
    p1 = make_proj_fillers(1, j, nxt1)
    f_wo = make_wo_fillers(j - 1)
    attn(0, j, 0, fillers=p1)
    attn(1, j, 0)
    finish(j, 0, AO)
    attn(0, j, 1, fillers=f_wo[:17])
    attn(1, j, 1)
    finish(j, 1, AO)
    # tail: leftover wo(2) runs immediately (independent of slab-3 A2As),
    # then the two-round wo(3); gathers go behind the collective triggers
    tail_wo = make_wo_fillers(j, tail=True)
    for f in f_wo[17:]:
        f()
    tail_wo[0]()                  # t0 gather (gpsimd, after both triggers)
    tail_wo[1]()                  # t1 gather
    for f in tail_wo[2:]:
        f()

    for p in (psO, psS, psM, dram, aogp, evac, pexp, rp, dsp, aop, qpool,
              xpool, kvp, wpool, const):
        p.release()


def _build(single=False):
    nc = bacc.Bacc("TRN2", target_bir_lowering=False, debug=False,
                   num_devices=1 if single else NCORES)
    io = {
        "xP": nc.dram_tensor("xP", [128, B * NSLAB * ND * QTILE], BF16,
                             kind="ExternalInput").ap(),
        "wq": nc.dram_tensor("wq", [128, 2 * ND * 128], BF16,
                             kind="ExternalInput").ap(),
        "wkv": nc.dram_tensor("wkv", [128, ND * 128], BF16,
                              kind="ExternalInput").ap(),
        "wo": nc.dram_tensor("wo", [128, ND * D], BF16,
                             kind="ExternalInput").ap(),
        "cos2": nc.dram_tensor("cos2", [128, S], BF16, kind="ExternalInput").ap(),
        "sin2s": nc.dram_tensor("sin2s", [128, S], BF16, kind="ExternalInput").ap(),
        "selc": nc.dram_tensor("selc", [QH, 4 * 64], BF16,
                               kind="ExternalInput").ap(),
        "tri": nc.dram_tensor("tri", [128, 2 * KTILE], BF16,
                              kind="ExternalInput").ap(),
        "out": nc.dram_tensor("out", [NSLAB * 128, D], F32,
                              kind="ExternalOutput").ap(),
    }
    io["single"] = single
    with tile.TileContext(nc) as tc:
        _build_kernel(tc, io)
    nc.compile()
    return nc


_CACHE = {}


def _get_program():
    if "nc" not in _CACHE:
        _CACHE["nc"] = _build()
    return _CACHE["nc"]


# head-dim permutation: pairs (d, d+32) end up 16 lanes apart within each
# 32-lane block, so rotate_half becomes a +16 lane rotation (stream_shuffle)
P64 = np.concatenate([np.arange(0, 16), np.arange(32, 48),
                      np.arange(16, 32), np.arange(48, 64)])


def _host_inputs(x, wq, wk, wv, wo):
    x = np.ascontiguousarray(x, np.float32)
    inv = 1.0 / (10000.0 ** (np.arange(0, HD, 2, dtype=np.float64) / HD))
    pos = np.arange(S, dtype=np.float64)
    freqs = np.outer(pos, inv)                      # [S, 32]
    emb = np.concatenate([freqs, freqs], axis=1)    # [S, 64]
    cos = np.cos(emb).T.astype(np.float32)          # [64, S]
    sin = np.sin(emb).T.astype(np.float32)
    sin2 = np.concatenate([-sin[:32], sin[32:]], axis=0)  # [64, S]
    cosP, sin2P = cos[P64], sin2[P64]
    cos2 = np.concatenate([cosP, cosP], axis=0)       # [128, S]
    sin2s = np.concatenate([sin2P, sin2P], axis=0)

    # denominator broadcast selector for finish(): picks dn row u = 2b+h
    selc = np.zeros((QH, 4 * 64), np.float32)
    for u in range(4):
        selc[u, u * 64:(u + 1) * 64] = 1.0

    # causal keep-mask for a 128x128 diagonal block (key=partition p kept
    # when local query c >= p), duplicated for the two heads
    tri1 = (np.arange(KTILE)[None, :] >= np.arange(KTILE)[:, None])
    tri = np.concatenate([tri1, tri1], axis=1).astype(np.float32)

    import ml_dtypes
    bf16 = ml_dtypes.bfloat16
    cos2 = cos2.astype(bf16)
    sin2s = sin2s.astype(bf16)
    selc = selc.astype(bf16)
    tri = tri.astype(bf16)

    # x prearranged so a (b, j) slab is one contiguous [128, 8192] column
    # block: xP[p, ((b*4+j)*16+k)*512+q] = x[b, j*512+q, k*128+p]
    xr = x.reshape(B, NSLAB, QTILE, ND, 128)
    xP = np.ascontiguousarray(
        xr.transpose(4, 0, 1, 3, 2).reshape(128, -1)).astype(bf16)

    # wo rows ordered to match the gathered A2A feature order (fc = 2cc+t,
    # within-chunk row 64h+p for head 4cc+t+2h), then prearranged so
    # partition p's data is contiguous: wo_l[p, k*2048+c] = wo_p[k*128+p, c]
    wrows = []
    for cc in range(NCORES):
        for t in range(2):
            for h in (4 * cc + t, 4 * cc + t + 2):
                wrows.append(wo[h * HD:(h + 1) * HD, :])
    wo_p = np.concatenate(wrows, axis=0)                     # [2048, 2048]
    wo_l = np.ascontiguousarray(
        wo_p.reshape(ND, 128, D).transpose(1, 0, 2).reshape(128, -1)
    ).astype(bf16)

    in_maps = []
    for c in range(NCORES):
        qcols = []
        for t in range(2):
            for h in (4 * c + t, 4 * c + t + 2):
                qcols.append(wq[:, h * HD:(h + 1) * HD][:, P64])
        wq_p = np.concatenate(qcols, axis=1)                 # [2048, 256]
        # wq_l[p, f*2048 + k*128 + cc] = wq_p[k*128+p, f*128+cc]
        wq_l = np.ascontiguousarray(
            wq_p.reshape(ND, 128, 2, 128).transpose(1, 2, 0, 3)
            .reshape(128, -1)).astype(bf16)
        wkv_p = np.concatenate(
            [wk[:, c * HD:(c + 1) * HD][:, P64],
             wv[:, c * HD:(c + 1) * HD]], axis=1)            # [2048, 128]
        wkv_l = np.ascontiguousarray(
            wkv_p.reshape(ND, 128, 128).transpose(1, 0, 2)
            .reshape(128, -1)).astype(bf16)
        in_maps.append({
            "xP": xP, "wq": wq_l, "wkv": wkv_l, "wo": wo_l,
            "cos2": cos2, "sin2s": sin2s, "selc": selc, "tri": tri,
        })
    return in_maps


def run(x, wq, wk, wv, wo, trace=False, **trace_kwargs):
    nc = _get_program()
    in_maps = _host_inputs(x, wq, wk, wv, wo)
    res = run_bass_kernel_spmd(nc, in_maps, list(range(NCORES)),
                               trace=trace, **trace_kwargs)
    out = np.empty((B, S, D), np.float32)
    for c in range(NCORES):
        bo, g = c // 4, c % 4
        shard = res.results[c]["out"]  # [512, D]
        for j in range(NSLAB):
            out[bo, j * QTILE + g * 128:j * QTILE + (g + 1) * 128, :] = \
                shard[j * 128:(j + 1) * 128, :]
    return out, res


def kernel(x, wq, wk, wv, wo):
    out, _ = run(x, wq, wk, wv, wo)
    return out.astype(np.float32)
